# revision 14
# baseline (speedup 1.0000x reference)
"""Trainium2 Bass kernel for CayleyCirculantSSMLayer (seq-sharded, 8 cores).

Math: lambda_j = e^{i*theta_j} with theta = -2*arctan(omega).  The causal conv
h[t] = sum_{s<=t} cos((t-s)theta) Bu[s] factors through a chunk-rotated
cumulative sum (chunks of L=256 rows): within a chunk A = Q + cumsum(rotBu),
h = cos(sigma*theta) (.) A_re - sin(sigma*theta) (.) A_im, carry
Q' = lambda^L (Q + R) chained across chunks.

Sharding: each core owns TWO quarter-sample pieces (sample k//2, quarters
k%2 and 2+k%2), so a sample's 4 quarters live on the core pair {2s, 2s+1}.
The causal carry entering quarter q is a linear combination of the
zero-initialized quarter totals T_r (r<q), exchanged between the pair via
tiny (4KB) AllGather collectives that hide under compute:

    A(p0) -> [allgather T(p0)] -> A(p1) -> [allgather T(p1)] -> B(p0) -> B(p1)

Phase A computes Bu (fp16 matmul), rotation, chunk cumsums (triangular
matmul), stashes locally-biased A in fp16, chains the local carry.  Phase B
applies the late carry correction delta_c = lambda^{256c} * Q_in as a
per-chunk bias, combines to h, projects through C (fp16 matmul), adds the
skip, and streams the output in fp16.  B-group prep (bias/combine) is
software-pipelined into the preceding emission so the PE never waits on it.
"""
import sys
import numpy as np

for p in ("/opt/trn_rl_repo",):
    if p not in sys.path:
        sys.path.insert(0, p)

from concourse import bass, bacc, mybir, tile
from concourse import bass_utils

D_MODEL = 1024
STATE = 512
BATCH = 4
SEQ = 8192
PIECE = 2048              # rows per piece (quarter sample)
L = 256                   # carry-chunk length
GL = 512                  # group length (2 chunks)
NGRP = PIECE // GL        # 4 groups per piece
NCHUNK = PIECE // L       # 8 chunks per piece
NT = 4                    # state n-tiles of 128
F32 = mybir.dt.float32
H = mybir.dt.float16
NPH = np.float16

_CACHE = {}


def build_nc():
    nc = bacc.Bacc(None, target_bir_lowering=False, num_devices=8)
    uT_d = nc.dram_tensor("uT", [D_MODEL, 2 * PIECE], H, kind="ExternalInput")
    bwT_d = nc.dram_tensor("bwT", [D_MODEL, STATE], H, kind="ExternalInput")
    cwT_d = nc.dram_tensor("cwT", [STATE, D_MODEL], H, kind="ExternalInput")
    rotc_d = nc.dram_tensor("rotc", [128, 4, STATE], H, kind="ExternalInput")
    cmb_d = nc.dram_tensor("cmb", [128, 2, NT, GL], H, kind="ExternalInput")
    U_d = nc.dram_tensor("U", [128, 128], H, kind="ExternalInput")
    chain_d = nc.dram_tensor("chain", [128, 2, NT], F32, kind="ExternalInput")
    rotc8_d = nc.dram_tensor("rotc8", [128, 2, NT, NCHUNK], F32, kind="ExternalInput")
    mask_d = nc.dram_tensor("mask", [128, 6, NT], F32, kind="ExternalInput")
    yT_d = nc.dram_tensor("yT", [D_MODEL, 2 * PIECE], H, kind="ExternalOutput")

    CP = mybir.ActivationFunctionType.Identity
    ADD = mybir.AluOpType.add
    GROUPS = [[0, 1], [2, 3], [4, 5], [6, 7]]

    with tile.TileContext(nc) as tc:
        with (
            tc.tile_pool(name="c", bufs=1) as cpool,
            tc.tile_pool(name="u", bufs=2) as upool,
            tc.tile_pool(name="q", bufs=3) as qpool,
            tc.tile_pool(name="dram", bufs=1, space="DRAM") as dpool,
            tc.tile_pool(name="psbu", bufs=2, space="PSUM") as psbu,
            tc.tile_pool(name="psw", bufs=4, space="PSUM") as psw,
            tc.tile_pool(name="psy", bufs=2, space="PSUM") as psy,
            tc.tile_pool(name="pa", bufs=1) as apool,
            tc.tile_pool(name="pv", bufs=3) as vpool,
            tc.tile_pool(name="pb", bufs=2) as bpool,
        ):
            # ---------- A-phase weights (gpsimd queue, loaded first) ----------
            bwT = []
            for k in range(8):
                t = apool.tile([128, STATE], H, tag=f"bw{k}", name=f"bw{k}")
                nc.gpsimd.dma_start(t[:], bwT_d[k * 128:(k + 1) * 128, :])
                bwT.append(t)
            rotc_t = apool.tile([128, 4, STATE], H, tag="rotc", name="rotc")
            nc.gpsimd.dma_start(rotc_t[:], rotc_d[:, :, :])
            U_t = apool.tile([128, 128], H, tag="U", name="U")
            nc.gpsimd.dma_start(U_t[:], U_d[:, :])
            chain_t = cpool.tile([128, 2, NT], F32, tag="chain", name="chain")
            nc.gpsimd.dma_start(chain_t[:], chain_d[:, :, :])
            mask_t = cpool.tile([128, 6, NT], F32, tag="mask", name="mask")
            nc.gpsimd.dma_start(mask_t[:], mask_d[:, :, :])

            # ---------- all u loads up front (SP queue) ----------
            uP = [[None] * 8, [None] * 8]
            for p in range(2):
                for k in range(8):
                    uP[p][k] = upool.tile([128, PIECE], H, tag=f"ut{k}",
                                          name=f"ut{k}")
                if p == 0:
                    # group-ordered small loads so A(0,g) starts ASAP
                    for g in range(NGRP):
                        for k in range(8):
                            nc.sync.dma_start(
                                uP[p][k][:, g * GL:(g + 1) * GL],
                                uT_d[k * 128:(k + 1) * 128,
                                     p * PIECE + g * GL:p * PIECE + (g + 1) * GL])
                else:
                    for k in range(8):
                        nc.sync.dma_start(
                            uP[p][k][:, 0:GL],
                            uT_d[k * 128:(k + 1) * 128, p * PIECE:p * PIECE + GL])
                    for k in range(8):
                        nc.sync.dma_start(
                            uP[p][k][:, GL:PIECE],
                            uT_d[k * 128:(k + 1) * 128,
                                 p * PIECE + GL:(p + 1) * PIECE])

            # ---------- persistent state ----------
            stash = [[cpool.tile([128, NT, PIECE], H, tag=f"st{p}{co}",
                                 name=f"st{p}{co}")
                      for co in range(2)] for p in range(2)]
            g_r0 = []
            g_r1 = []
            for p in range(2):
                g_r0.append(cpool.tile([128, 8], F32, tag=f"gr0{p}", name=f"gr0{p}"))
                g_r1.append(cpool.tile([128, 8], F32, tag=f"gr1{p}", name=f"gr1{p}"))
            cwT = []
            cmb_t = None
            rotc8_t = None
            qfin = [None, None]          # final chain tiles per piece
            qin = [None, None]           # (re, im) per piece
            hts = {}                     # (p, g) -> hT3 tile

            # ================= emission helpers =================
            def emit_A_group(p, g, q):
                v = []
                for s4 in range(4):
                    bu_ps = psbu.tile([128, STATE], F32, tag="bu", name="bu")
                    for k in range(8):
                        nc.tensor.matmul(
                            bu_ps[:],
                            uP[p][k][:, g * GL + s4 * 128:g * GL + (s4 + 1) * 128],
                            bwT[k][:],
                            start=(k == 0), stop=(k == 7))
                    sub = s4 % 2
                    vre = vpool.tile([128, STATE], H, tag="vre", name="vre")
                    nc.vector.tensor_mul(vre[:], bu_ps[:], rotc_t[:, sub, :])
                    vim = vpool.tile([128, STATE], H, tag="vim", name="vim")
                    nc.vector.tensor_mul(vim[:], bu_ps[:], rotc_t[:, 2 + sub, :])
                    v.append((vre, vim))

                for c in range(2):
                    ci = 2 * g + c
                    x0, x1 = v[2 * c], v[2 * c + 1]
                    tc_re = qpool.tile([128, NT], F32, tag="tcre", name="tcre")
                    tc_im = qpool.tile([128, NT], F32, tag="tcim", name="tcim")
                    tcs = [tc_re, tc_im]
                    for co in range(2):
                        d0, d1 = x0[co], x1[co]
                        for pr in range(2):
                            w_ps = psw.tile([128, GL], F32, tag="w", name="w")
                            for half in range(2):
                                ntile = 2 * pr + half
                                base = half * 256
                                nc.tensor.matmul(
                                    w_ps[:, base:base + 128],
                                    d0[:, ntile * 128:(ntile + 1) * 128],
                                    U_t[:], start=True, stop=True)
                                nc.tensor.matmul(
                                    w_ps[:, base + 128:base + 256],
                                    d1[:, ntile * 128:(ntile + 1) * 128],
                                    U_t[:], start=True, stop=True)
                            # block-1 bias = q + colsum(v0) = q + w[col 127]
                            b2 = qpool.tile([128, 2], F32, tag="b2", name="b2",
                                            bufs=4)
                            nc.vector.tensor_add(
                                b2[:], w_ps[:, 127::256],
                                q[co][:, 2 * pr:2 * pr + 2])
                            for half in range(2):
                                ntile = 2 * pr + half
                                base = half * 256
                                nc.scalar.activation(
                                    stash[p][co][:, ntile,
                                                 ci * 256:ci * 256 + 128],
                                    w_ps[:, base:base + 128],
                                    CP, bias=q[co][:, ntile:ntile + 1])
                                nc.scalar.activation(
                                    stash[p][co][:, ntile,
                                                 ci * 256 + 128:ci * 256 + 256],
                                    w_ps[:, base + 128:base + 256],
                                    CP, bias=b2[:, half:half + 1])
                            nc.vector.tensor_add(
                                tcs[co][:, 2 * pr:2 * pr + 2],
                                w_ps[:, 255::256],
                                b2[:])
                    m1 = qpool.tile([128, NT], F32, tag="m1", name="m1")
                    nc.vector.tensor_mul(m1[:], tc_re[:], chain_t[:, 0, :])
                    m2 = qpool.tile([128, NT], F32, tag="m2", name="m2")
                    nc.vector.tensor_mul(m2[:], tc_im[:], chain_t[:, 1, :])
                    qre = qpool.tile([128, NT], F32, tag="qre", name="qre")
                    nc.vector.tensor_sub(qre[:], m1[:], m2[:])
                    m3 = qpool.tile([128, NT], F32, tag="m3", name="m3")
                    nc.vector.tensor_mul(m3[:], tc_im[:], chain_t[:, 0, :])
                    m4 = qpool.tile([128, NT], F32, tag="m4", name="m4")
                    nc.vector.tensor_mul(m4[:], tc_re[:], chain_t[:, 1, :])
                    qim = qpool.tile([128, NT], F32, tag="qim", name="qim")
                    nc.vector.tensor_add(qim[:], m3[:], m4[:])
                    q = [qre, qim]
                return q

            def emit_coll_pack(p):
                send_sb = qpool.tile([128, 8], F32, tag="send", name="send")
                nc.vector.tensor_copy(send_sb[:, 0:4], qfin[p][0][:])
                nc.vector.tensor_copy(send_sb[:, 4:8], qfin[p][1][:])
                return send_sb

            def emit_coll_comm(p, send_sb):
                send_d = dpool.tile([128, 8], F32, tag=f"send{p}", name=f"send{p}")
                recv_d = dpool.tile([256, 8], F32, tag=f"recv{p}", name=f"recv{p}")
                nc.gpsimd.dma_start(send_d[:], send_sb[:])
                nc.gpsimd.collective_compute(
                    "AllGather",
                    mybir.AluOpType.bypass,
                    replica_groups=GROUPS,
                    ins=[send_d[:].opt()],
                    outs=[recv_d[:].opt()],
                )
                return recv_d

            def emit_recv_sbuf(p, recv_d, eng):
                eng.dma_start(g_r0[p][:], recv_d[0:128, :])
                eng.dma_start(g_r1[p][:], recv_d[128:256, :])

            def emit_B_weights():
                for k in range(4):
                    t = cpool.tile([128, D_MODEL], H, tag=f"cw{k}", name=f"cw{k}")
                    nc.gpsimd.dma_start(t[:], cwT_d[k * 128:(k + 1) * 128, :])
                    cwT.append(t)
                cmb = cpool.tile([128, 2, NT, GL], H, tag="cmb", name="cmb")
                nc.gpsimd.dma_start(cmb[:], cmb_d[:, :, :, :])
                r8 = cpool.tile([128, 2, NT, NCHUNK], F32, tag="rotc8", name="rotc8")
                nc.gpsimd.dma_start(r8[:], rotc8_d[:, :, :, :])
                return cmb, r8

            def emit_qin(p):
                qinre = qpool.tile([128, NT], F32, tag="qinre", name="qinre")
                qinim = qpool.tile([128, NT], F32, tag="qinim", name="qinim")
                if p == 0:
                    nc.vector.tensor_mul(qinre[:], mask_t[:, 0, :], g_r0[0][:, 0:4])
                    nc.vector.tensor_mul(qinim[:], mask_t[:, 0, :], g_r0[0][:, 4:8])
                else:
                    acc_re = None
                    acc_im = None
                    terms = [
                        (1, g_r0[1], 0, 1.0), (2, g_r1[0], 0, 1.0),
                        (3, g_r1[0], 4, -1.0), (4, g_r0[0], 0, 1.0),
                        (5, g_r0[0], 4, -1.0),
                    ]
                    # re part: b*G2re + W1re*G1[1]re - W1im*G1[1]im
                    #          + W2re*G1[0]re - W2im*G1[0]im
                    for i, (mi, src, off, sgn) in enumerate(terms):
                        t = qpool.tile([128, NT], F32, tag=f"x{i}", name=f"x{i}")
                        nc.vector.tensor_mul(t[:], mask_t[:, mi, :],
                                             src[:, off:off + 4])
                        if acc_re is None:
                            acc_re = t
                        else:
                            nt_ = qpool.tile([128, NT], F32, tag=f"xa{i}",
                                             name=f"xa{i}")
                            if sgn > 0:
                                nc.vector.tensor_add(nt_[:], acc_re[:], t[:])
                            else:
                                nc.vector.tensor_sub(nt_[:], acc_re[:], t[:])
                            acc_re = nt_
                    terms_im = [
                        (1, g_r0[1], 4), (2, g_r1[0], 4), (3, g_r1[0], 0),
                        (4, g_r0[0], 4), (5, g_r0[0], 0),
                    ]
                    for i, (mi, src, off) in enumerate(terms_im):
                        t = qpool.tile([128, NT], F32, tag=f"y{i}", name=f"y{i}")
                        nc.vector.tensor_mul(t[:], mask_t[:, mi, :],
                                             src[:, off:off + 4])
                        if acc_im is None:
                            acc_im = t
                        else:
                            nt_ = qpool.tile([128, NT], F32, tag=f"ya{i}",
                                             name=f"ya{i}")
                            nc.vector.tensor_add(nt_[:], acc_im[:], t[:])
                            acc_im = nt_
                    nc.vector.tensor_copy(qinre[:], acc_re[:])
                    nc.vector.tensor_copy(qinim[:], acc_im[:])
                qin[p] = (qinre, qinim)
                return qinre, qinim

            deltas = {}

            def emit_prep_delta(p, g):
                # delta_c = lam^{256 ci} * Q_in, on gpsimd (Pool)
                qinre, qinim = qin[p]
                for c in range(2):
                    ci = 2 * g + c
                    dre = qpool.tile([128, NT], F32, tag="dre", name="dre", bufs=4)
                    dim = qpool.tile([128, NT], F32, tag="dim", name="dim", bufs=4)
                    e1 = qpool.tile([128, NT], F32, tag="e1", name="e1")
                    nc.gpsimd.tensor_mul(e1[:], rotc8_t[:, 0, :, ci], qinre[:])
                    e2 = qpool.tile([128, NT], F32, tag="e2", name="e2")
                    nc.gpsimd.tensor_mul(e2[:], rotc8_t[:, 1, :, ci], qinim[:])
                    nc.gpsimd.tensor_sub(dre[:], e1[:], e2[:])
                    e3 = qpool.tile([128, NT], F32, tag="e3", name="e3")
                    nc.gpsimd.tensor_mul(e3[:], rotc8_t[:, 0, :, ci], qinim[:])
                    e4 = qpool.tile([128, NT], F32, tag="e4", name="e4")
                    nc.gpsimd.tensor_mul(e4[:], rotc8_t[:, 1, :, ci], qinre[:])
                    nc.gpsimd.tensor_add(dim[:], e3[:], e4[:])
                    deltas[(p, ci)] = (dre, dim)

            def emit_prep_bias(p, g):
                # a3 = stash + delta; split: re nt0-1 DVE, re nt2-3 Act, im Pool
                a3re = bpool.tile([128, NT, GL], H, tag="a3re", name="a3re", bufs=1)
                a3im = bpool.tile([128, NT, GL], H, tag="a3im", name="a3im", bufs=1)
                for c in range(2):
                    ci = 2 * g + c
                    dre, dim = deltas[(p, ci)]
                    cr = slice(c * 256, (c + 1) * 256)
                    for ntile in range(NT):
                        st = stash[p][0][:, ntile, ci * 256:(ci + 1) * 256]
                        if ntile < 2:
                            nc.vector.tensor_scalar(
                                a3re[:, ntile, cr], st,
                                dre[:, ntile:ntile + 1], None, ADD)
                        else:
                            nc.scalar.activation(
                                a3re[:, ntile, cr], st, CP,
                                bias=dre[:, ntile:ntile + 1])
                    for ntile in range(NT):
                        st = stash[p][1][:, ntile, ci * 256:(ci + 1) * 256]
                        nc.gpsimd.tensor_scalar(
                            a3im[:, ntile, cr], st,
                            dim[:, ntile:ntile + 1], None, ADD)
                return a3re, a3im

            def emit_prep_comb(p, g, a3re, a3im):
                p1 = bpool.tile([128, NT, GL], H, tag="p1", name="p1", bufs=1)
                nc.vector.tensor_mul(p1[:], a3re[:], cmb_t[:, 0, :, :])
                p2 = bpool.tile([128, NT, GL], H, tag="p2", name="p2", bufs=1)
                nc.vector.tensor_mul(p2[:], a3im[:], cmb_t[:, 1, :, :])
                hT3 = bpool.tile([128, NT, GL], H, tag="h", name="hT3", bufs=3)
                nc.vector.tensor_sub(hT3[:], p1[:], p2[:])
                hts[(p, g)] = hT3

            def emit_B_proj(p, g):
                hT3 = hts.pop((p, g))
                for mt in range(8):
                    y_ps = psy.tile([128, GL], F32, tag="y", name="y_ps")
                    for kt in range(4):
                        nc.tensor.matmul(
                            y_ps[:],
                            cwT[kt][:, mt * 128:(mt + 1) * 128],
                            hT3[:, kt, :],
                            start=(kt == 0), stop=(kt == 3))
                    yc = bpool.tile([128, GL], H, tag="yc", name="yc", bufs=4)
                    nc.scalar.activation(yc[:], y_ps[:], CP)
                    yo = bpool.tile([128, GL], H, tag="yo", name="yo", bufs=6)
                    nc.vector.tensor_add(yo[:], yc[:],
                                         uP[p][mt][:, g * GL:(g + 1) * GL])
                    nc.sync.dma_start(
                        yT_d[mt * 128:(mt + 1) * 128,
                             p * PIECE + g * GL:p * PIECE + (g + 1) * GL],
                        yo[:])

            # ================= emission schedule =================
            def zero_q():
                qre = qpool.tile([128, NT], F32, tag="qre", name="qre")
                nc.vector.memset(qre[:], 0.0)
                qim = qpool.tile([128, NT], F32, tag="qim", name="qim")
                nc.vector.memset(qim[:], 0.0)
                return [qre, qim]

            q = zero_q()
            for g in range(NGRP):
                q = emit_A_group(0, g, q)
            qfin[0] = q
            sb0 = emit_coll_pack(0)
            rd0 = emit_coll_comm(0, sb0)
            emit_recv_sbuf(0, rd0, nc.gpsimd)
            cmb_t, rotc8_t = emit_B_weights()

            q = zero_q()
            q = emit_A_group(1, 0, q)
            q = emit_A_group(1, 1, q)
            emit_qin(0)
            emit_prep_delta(0, 0)
            q = emit_A_group(1, 2, q)
            a3_00 = emit_prep_bias(0, 0)
            emit_prep_delta(0, 1)
            q = emit_A_group(1, 3, q)
            qfin[1] = q
            sb1 = emit_coll_pack(1)
            emit_prep_comb(0, 0, *a3_00)
            a3_01 = emit_prep_bias(0, 1)
            rd1 = emit_coll_comm(1, sb1)
            emit_prep_comb(0, 1, *a3_01)
            emit_B_proj(0, 0)
            emit_prep_delta(0, 2)
            a3 = emit_prep_bias(0, 2)
            emit_prep_comb(0, 2, *a3)
            emit_B_proj(0, 1)
            emit_recv_sbuf(1, rd1, nc.sync)
            emit_prep_delta(0, 3)
            a3 = emit_prep_bias(0, 3)
            emit_prep_comb(0, 3, *a3)
            emit_B_proj(0, 2)
            emit_qin(1)
            emit_prep_delta(1, 0)
            a3 = emit_prep_bias(1, 0)
            emit_prep_comb(1, 0, *a3)
            emit_B_proj(0, 3)
            emit_prep_delta(1, 1)
            a3 = emit_prep_bias(1, 1)
            emit_prep_comb(1, 1, *a3)
            emit_B_proj(1, 0)
            emit_prep_delta(1, 2)
            a3 = emit_prep_bias(1, 2)
            emit_prep_comb(1, 2, *a3)
            emit_B_proj(1, 1)
            emit_prep_delta(1, 3)
            a3 = emit_prep_bias(1, 3)
            emit_prep_comb(1, 3, *a3)
            emit_B_proj(1, 2)
            emit_B_proj(1, 3)
    nc.compile()
    return nc


def _host_tables(a_params):
    n = STATE
    half = n // 2
    a_full = np.zeros(n)
    a_full[1:half + 1] = a_params.astype(np.float64)
    a_full[half + 1:] = -a_params.astype(np.float64)[::-1][: n - half - 1]
    omega = np.imag(np.fft.fft(a_full))
    theta = -2.0 * np.arctan(omega)          # (512,)
    p128 = np.arange(128)

    c0Sa = np.cos(p128[:, None] * theta[None, :])
    c0Sb = np.cos((p128[:, None] + 128) * theta[None, :])
    ms0Sa = -np.sin(p128[:, None] * theta[None, :])
    ms0Sb = -np.sin((p128[:, None] + 128) * theta[None, :])
    rotc = np.stack([c0Sa, c0Sb, ms0Sa, ms0Sb], axis=1)       # (128, 4, 512)

    tg = np.arange(GL) % 256
    cmb = np.empty((128, 2, NT, GL))
    for nt in range(NT):
        th = theta[128 * nt:128 * (nt + 1)]
        cmb[:, 0, nt, :] = np.cos(th[:, None] * tg[None, :])
        cmb[:, 1, nt, :] = np.sin(th[:, None] * tg[None, :])

    U = np.triu(np.ones((128, 128)))

    thNT = theta.reshape(NT, 128).T                            # (128, NT)
    chain = np.stack([np.cos(L * thNT), np.sin(L * thNT)], axis=1)

    cs = np.arange(NCHUNK)
    ang8 = thNT[:, :, None] * (cs[None, None, :] * 256.0)      # (128, NT, 8)
    rotc8 = np.stack([np.cos(ang8), np.sin(ang8)], axis=1)     # (128, 2, NT, 8)

    rho = np.exp(1j * 2048.0 * thNT)                           # (128, NT) complex
    tabs = {
        "rotc": rotc.astype(NPH),
        "cmb": cmb.astype(NPH),
        "U": U.astype(NPH),
        "chain": np.ascontiguousarray(chain, dtype=np.float32),
        "rotc8": np.ascontiguousarray(rotc8, dtype=np.float32),
    }
    return tabs, rho


def _masks_for(member, rho):
    ones = np.ones_like(rho.real)
    zeros = np.zeros_like(rho.real)
    if member == 0:
        m0, b = zeros, zeros
        W1, W2 = ones + 0j, rho
    else:
        m0, b = ones, ones
        W1, W2 = rho, rho * rho
    mask = np.stack([m0, b, W1.real, W1.imag, W2.real, W2.imag], axis=1)
    return np.ascontiguousarray(mask, dtype=np.float32)        # (128, 6, NT)


def kernel(u, a_params, B_w, C_w, D, trace=False):
    u = np.asarray(u, dtype=np.float32)
    B_w = np.asarray(B_w, dtype=np.float32)
    C_w = np.asarray(C_w, dtype=np.float32)
    D = np.asarray(D, dtype=np.float32)
    assert np.allclose(D, 1.0), "kernel assumes D == ones (skip shares u tiles)"
    tabs, rho = _host_tables(np.asarray(a_params))

    if "nc" not in _CACHE:
        _CACHE["nc"] = build_nc()
    nc = _CACHE["nc"]

    bwT = np.ascontiguousarray(B_w.T).astype(NPH)              # (1024, 512)
    cwT = np.ascontiguousarray(C_w.T).astype(NPH)              # (512, 1024)

    in_maps = []
    for core in range(8):
        s, m = core // 2, core % 2
        qa, qb = m, 2 + m
        uT = np.concatenate(
            [u[s, qa * PIECE:(qa + 1) * PIECE, :].T,
             u[s, qb * PIECE:(qb + 1) * PIECE, :].T], axis=1)   # (1024, 4096)
        mp = {
            "uT": np.ascontiguousarray(uT).astype(NPH),
            "bwT": bwT,
            "cwT": cwT,
            "mask": _masks_for(m, rho),
        }
        mp.update(tabs)
        in_maps.append(mp)

    res = bass_utils.run_bass_kernel_spmd(
        nc, in_maps, core_ids=list(range(8)), trace=trace)
    y = np.empty((BATCH, SEQ, D_MODEL), dtype=np.float32)
    for core in range(8):
        s, m = core // 2, core % 2
        qa, qb = m, 2 + m
        yT = res.results[core]["yT"].astype(np.float32)        # (1024, 4096) fp16
        y[s, qa * PIECE:(qa + 1) * PIECE, :] = yT[:, 0:PIECE].T
        y[s, qb * PIECE:(qb + 1) * PIECE, :] = yT[:, PIECE:2 * PIECE].T
    _CACHE["last_res"] = res
    return y


# revision 19
# speedup vs baseline: 1.1728x; 1.1728x over previous
"""Trainium2 Bass kernel for CayleyCirculantSSMLayer (seq-sharded, 8 cores).

Math: lambda_j = e^{i*theta_j} with theta = -2*arctan(omega).  The causal conv
h[t] = sum_{s<=t} cos((t-s)theta) Bu[s] factors through a chunk-rotated
cumulative sum (chunks of L=256 rows): within a chunk A = Q + cumsum(rotBu),
h = cos(sigma*theta) (.) A_re - sin(sigma*theta) (.) A_im, carry
Q' = lambda^L (Q + R) chained across chunks.

Sharding: each core owns TWO quarter-sample pieces (sample k//2, quarters
k%2 and 2+k%2), so a sample's 4 quarters live on the core pair {2s, 2s+1}.
The causal carry entering quarter q is a linear combination of the
zero-initialized quarter totals T_r (r<q), exchanged between the pair via
tiny (4KB) AllGather collectives that hide under compute:

    A(p0) -> [allgather T(p0)] -> A(p1) -> [allgather T(p1)] -> B(p0) -> B(p1)

Phase A computes Bu (fp16 matmul), rotation, chunk cumsums (triangular
matmul), stashes locally-biased A in fp16, chains the local carry.  Phase B
applies the late carry correction delta_c = lambda^{256c} * Q_in as a
per-chunk bias, combines to h, projects through C (fp16 matmul), adds the
skip, and streams the output in fp16.  B-group prep (bias/combine) is
software-pipelined into the preceding emission so the PE never waits on it.
"""
import sys
import numpy as np

for p in ("/opt/trn_rl_repo",):
    if p not in sys.path:
        sys.path.insert(0, p)

from concourse import bass, bacc, mybir, tile
from concourse import bass_utils

D_MODEL = 1024
STATE = 512
BATCH = 4
SEQ = 8192
PIECE = 2048              # rows per piece (quarter sample)
L = 256                   # carry-chunk length
GL = 512                  # group length (2 chunks)
NGRP = PIECE // GL        # 4 groups per piece
NCHUNK = PIECE // L       # 8 chunks per piece
NT = 4                    # state n-tiles of 128
F32 = mybir.dt.float32
H = mybir.dt.float16
NPH = np.float16

_CACHE = {}


def build_nc():
    nc = bacc.Bacc(None, target_bir_lowering=False, num_devices=8)
    uT_d = nc.dram_tensor("uT", [D_MODEL, 2 * PIECE], H, kind="ExternalInput")
    bwT_d = nc.dram_tensor("bwT", [D_MODEL, STATE], H, kind="ExternalInput")
    cwT_d = nc.dram_tensor("cwT", [STATE, D_MODEL], H, kind="ExternalInput")
    rotc_d = nc.dram_tensor("rotc", [128, 4, STATE], H, kind="ExternalInput")
    cmb_d = nc.dram_tensor("cmb", [128, 2, NT, GL], H, kind="ExternalInput")
    UO_d = nc.dram_tensor("UO", [128, 256], H, kind="ExternalInput")
    ZU_d = nc.dram_tensor("ZU", [128, 256], H, kind="ExternalInput")
    chain_d = nc.dram_tensor("chain", [128, 2, NT], F32, kind="ExternalInput")
    rotc8_d = nc.dram_tensor("rotc8", [128, 2, NT, NCHUNK], F32, kind="ExternalInput")
    mask_d = nc.dram_tensor("mask", [128, 6, NT], F32, kind="ExternalInput")
    yT_d = nc.dram_tensor("yT", [D_MODEL, 2 * PIECE], H, kind="ExternalOutput")

    CP = mybir.ActivationFunctionType.Identity
    ADD = mybir.AluOpType.add
    GROUPS = [[0, 1], [2, 3], [4, 5], [6, 7]]

    with tile.TileContext(nc) as tc:
        with (
            tc.tile_pool(name="c", bufs=1) as cpool,
            tc.tile_pool(name="u", bufs=2) as upool,
            tc.tile_pool(name="q", bufs=3) as qpool,
            tc.tile_pool(name="dram", bufs=1, space="DRAM") as dpool,
            tc.tile_pool(name="psbu", bufs=2, space="PSUM") as psbu,
            tc.tile_pool(name="psw", bufs=4, space="PSUM") as psw,
            tc.tile_pool(name="psy", bufs=2, space="PSUM") as psy,
            tc.tile_pool(name="pa", bufs=1) as apool,
            tc.tile_pool(name="pv", bufs=3) as vpool,
            tc.tile_pool(name="pb", bufs=2) as bpool,
        ):
            # ---------- A-phase weights (gpsimd queue, loaded first) ----------
            bwT = []
            for k in range(8):
                t = apool.tile([128, STATE], H, tag=f"bw{k}", name=f"bw{k}")
                nc.gpsimd.dma_start(t[:], bwT_d[k * 128:(k + 1) * 128, :])
                bwT.append(t)
            rotc_t = apool.tile([128, 4, STATE], H, tag="rotc", name="rotc")
            nc.gpsimd.dma_start(rotc_t[:], rotc_d[:, :, :])
            UO_t = apool.tile([128, 256], H, tag="UO", name="UO")
            nc.gpsimd.dma_start(UO_t[:], UO_d[:, :])
            ZU_t = apool.tile([128, 256], H, tag="ZU", name="ZU")
            nc.gpsimd.dma_start(ZU_t[:], ZU_d[:, :])
            chain_t = cpool.tile([128, 2, NT], F32, tag="chain", name="chain")
            nc.gpsimd.dma_start(chain_t[:], chain_d[:, :, :])
            mask_t = cpool.tile([128, 6, NT], F32, tag="mask", name="mask")
            nc.gpsimd.dma_start(mask_t[:], mask_d[:, :, :])

            # ---------- all u loads up front (SP queue) ----------
            uP = [[None] * 8, [None] * 8]
            for p in range(2):
                for k in range(8):
                    uP[p][k] = upool.tile([128, PIECE], H, tag=f"ut{k}",
                                          name=f"ut{k}")
                if p == 0:
                    # group-ordered small loads so A(0,g) starts ASAP
                    for g in range(NGRP):
                        for k in range(8):
                            nc.sync.dma_start(
                                uP[p][k][:, g * GL:(g + 1) * GL],
                                uT_d[k * 128:(k + 1) * 128,
                                     p * PIECE + g * GL:p * PIECE + (g + 1) * GL])
                else:
                    for k in range(8):
                        nc.sync.dma_start(
                            uP[p][k][:, 0:GL],
                            uT_d[k * 128:(k + 1) * 128, p * PIECE:p * PIECE + GL])
                    for k in range(8):
                        nc.sync.dma_start(
                            uP[p][k][:, GL:PIECE],
                            uT_d[k * 128:(k + 1) * 128,
                                 p * PIECE + GL:(p + 1) * PIECE])

            # ---------- persistent state ----------
            stash = [[cpool.tile([128, NT, PIECE], H, tag=f"st{p}{co}",
                                 name=f"st{p}{co}")
                      for co in range(2)] for p in range(2)]
            g_r0 = []
            g_r1 = []
            for p in range(2):
                g_r0.append(cpool.tile([128, 8], F32, tag=f"gr0{p}", name=f"gr0{p}"))
                g_r1.append(cpool.tile([128, 8], F32, tag=f"gr1{p}", name=f"gr1{p}"))
            cwT = []
            cmb_t = None
            rotc8_t = None
            qfin = [None, None]          # final chain tiles per piece
            qin = [None, None]           # (re, im) per piece
            hts = {}                     # (p, g) -> hT3 tile

            # ================= emission helpers =================
            def emit_A_group(p, g, q):
                v = []
                for s4 in range(4):
                    bu_ps = psbu.tile([128, STATE], F32, tag="bu", name="bu")
                    for k in range(8):
                        nc.tensor.matmul(
                            bu_ps[:],
                            uP[p][k][:, g * GL + s4 * 128:g * GL + (s4 + 1) * 128],
                            bwT[k][:],
                            start=(k == 0), stop=(k == 7))
                    sub = s4 % 2
                    vre = vpool.tile([128, STATE], H, tag="vre", name="vre")
                    nc.vector.tensor_mul(vre[:], bu_ps[:], rotc_t[:, sub, :])
                    vim = vpool.tile([128, STATE], H, tag="vim", name="vim")
                    nc.vector.tensor_mul(vim[:], bu_ps[:], rotc_t[:, 2 + sub, :])
                    v.append((vre, vim))

                for c in range(2):
                    ci = 2 * g + c
                    x0, x1 = v[2 * c], v[2 * c + 1]
                    tc_re = qpool.tile([128, NT], F32, tag="tcre", name="tcre")
                    tc_im = qpool.tile([128, NT], F32, tag="tcim", name="tcim")
                    tcs = [tc_re, tc_im]
                    for co in range(2):
                        d0, d1 = x0[co], x1[co]
                        for pr in range(2):
                            w_ps = psw.tile([128, GL], F32, tag="w", name="w")
                            for half in range(2):
                                ntile = 2 * pr + half
                                colr = slice(half * 256, half * 256 + 256)
                                nc.tensor.matmul(
                                    w_ps[:, colr],
                                    d0[:, ntile * 128:(ntile + 1) * 128],
                                    UO_t[:], start=True, stop=False)
                                nc.tensor.matmul(
                                    w_ps[:, colr],
                                    d1[:, ntile * 128:(ntile + 1) * 128],
                                    ZU_t[:], start=False, stop=True)
                            for half in range(2):
                                ntile = 2 * pr + half
                                nc.scalar.activation(
                                    stash[p][co][:, ntile, ci * 256:(ci + 1) * 256],
                                    w_ps[:, half * 256:half * 256 + 256],
                                    CP, bias=q[co][:, ntile:ntile + 1])
                            nc.vector.tensor_add(
                                tcs[co][:, 2 * pr:2 * pr + 2],
                                w_ps[:, 255::256],
                                q[co][:, 2 * pr:2 * pr + 2])
                    m1 = qpool.tile([128, NT], F32, tag="m1", name="m1")
                    nc.vector.tensor_mul(m1[:], tc_re[:], chain_t[:, 0, :])
                    m2 = qpool.tile([128, NT], F32, tag="m2", name="m2")
                    nc.vector.tensor_mul(m2[:], tc_im[:], chain_t[:, 1, :])
                    qre = qpool.tile([128, NT], F32, tag="qre", name="qre")
                    nc.vector.tensor_sub(qre[:], m1[:], m2[:])
                    m3 = qpool.tile([128, NT], F32, tag="m3", name="m3")
                    nc.vector.tensor_mul(m3[:], tc_im[:], chain_t[:, 0, :])
                    m4 = qpool.tile([128, NT], F32, tag="m4", name="m4")
                    nc.vector.tensor_mul(m4[:], tc_re[:], chain_t[:, 1, :])
                    qim = qpool.tile([128, NT], F32, tag="qim", name="qim")
                    nc.vector.tensor_add(qim[:], m3[:], m4[:])
                    q = [qre, qim]
                return q

            def emit_coll_pack(p):
                send_sb = qpool.tile([128, 8], F32, tag="send", name="send")
                nc.vector.tensor_copy(send_sb[:, 0:4], qfin[p][0][:])
                nc.vector.tensor_copy(send_sb[:, 4:8], qfin[p][1][:])
                return send_sb

            def emit_coll_comm(p, send_sb):
                send_d = dpool.tile([128, 8], F32, tag=f"send{p}", name=f"send{p}")
                recv_d = dpool.tile([256, 8], F32, tag=f"recv{p}", name=f"recv{p}")
                nc.gpsimd.dma_start(send_d[:], send_sb[:])
                nc.gpsimd.collective_compute(
                    "AllGather",
                    mybir.AluOpType.bypass,
                    replica_groups=GROUPS,
                    ins=[send_d[:].opt()],
                    outs=[recv_d[:].opt()],
                )
                return recv_d

            def emit_recv_sbuf(p, recv_d, eng):
                eng.dma_start(g_r0[p][:], recv_d[0:128, :])
                eng.dma_start(g_r1[p][:], recv_d[128:256, :])

            def emit_B_weights():
                for k in range(4):
                    t = cpool.tile([128, D_MODEL], H, tag=f"cw{k}", name=f"cw{k}")
                    nc.gpsimd.dma_start(t[:], cwT_d[k * 128:(k + 1) * 128, :])
                    cwT.append(t)
                cmb = cpool.tile([128, 2, NT, GL], H, tag="cmb", name="cmb")
                nc.gpsimd.dma_start(cmb[:], cmb_d[:, :, :, :])
                r8 = cpool.tile([128, 2, NT, NCHUNK], F32, tag="rotc8", name="rotc8")
                nc.gpsimd.dma_start(r8[:], rotc8_d[:, :, :, :])
                return cmb, r8

            def emit_qin(p):
                qinre = qpool.tile([128, NT], F32, tag="qinre", name="qinre")
                qinim = qpool.tile([128, NT], F32, tag="qinim", name="qinim")
                if p == 0:
                    nc.vector.tensor_mul(qinre[:], mask_t[:, 0, :], g_r0[0][:, 0:4])
                    nc.vector.tensor_mul(qinim[:], mask_t[:, 0, :], g_r0[0][:, 4:8])
                else:
                    acc_re = None
                    acc_im = None
                    terms = [
                        (1, g_r0[1], 0, 1.0), (2, g_r1[0], 0, 1.0),
                        (3, g_r1[0], 4, -1.0), (4, g_r0[0], 0, 1.0),
                        (5, g_r0[0], 4, -1.0),
                    ]
                    # re part: b*G2re + W1re*G1[1]re - W1im*G1[1]im
                    #          + W2re*G1[0]re - W2im*G1[0]im
                    for i, (mi, src, off, sgn) in enumerate(terms):
                        t = qpool.tile([128, NT], F32, tag=f"x{i}", name=f"x{i}")
                        nc.vector.tensor_mul(t[:], mask_t[:, mi, :],
                                             src[:, off:off + 4])
                        if acc_re is None:
                            acc_re = t
                        else:
                            nt_ = qpool.tile([128, NT], F32, tag=f"xa{i}",
                                             name=f"xa{i}")
                            if sgn > 0:
                                nc.vector.tensor_add(nt_[:], acc_re[:], t[:])
                            else:
                                nc.vector.tensor_sub(nt_[:], acc_re[:], t[:])
                            acc_re = nt_
                    terms_im = [
                        (1, g_r0[1], 4), (2, g_r1[0], 4), (3, g_r1[0], 0),
                        (4, g_r0[0], 4), (5, g_r0[0], 0),
                    ]
                    for i, (mi, src, off) in enumerate(terms_im):
                        t = qpool.tile([128, NT], F32, tag=f"y{i}", name=f"y{i}")
                        nc.vector.tensor_mul(t[:], mask_t[:, mi, :],
                                             src[:, off:off + 4])
                        if acc_im is None:
                            acc_im = t
                        else:
                            nt_ = qpool.tile([128, NT], F32, tag=f"ya{i}",
                                             name=f"ya{i}")
                            nc.vector.tensor_add(nt_[:], acc_im[:], t[:])
                            acc_im = nt_
                    nc.vector.tensor_copy(qinre[:], acc_re[:])
                    nc.vector.tensor_copy(qinim[:], acc_im[:])
                qin[p] = (qinre, qinim)
                return qinre, qinim

            deltas = {}

            def emit_prep_delta(p, g):
                # delta_c = lam^{256 ci} * Q_in, on gpsimd (Pool)
                qinre, qinim = qin[p]
                for c in range(2):
                    ci = 2 * g + c
                    dre = qpool.tile([128, NT], F32, tag="dre", name="dre", bufs=4)
                    dim = qpool.tile([128, NT], F32, tag="dim", name="dim", bufs=4)
                    e1 = qpool.tile([128, NT], F32, tag="e1", name="e1")
                    nc.gpsimd.tensor_mul(e1[:], rotc8_t[:, 0, :, ci], qinre[:])
                    e2 = qpool.tile([128, NT], F32, tag="e2", name="e2")
                    nc.gpsimd.tensor_mul(e2[:], rotc8_t[:, 1, :, ci], qinim[:])
                    nc.gpsimd.tensor_sub(dre[:], e1[:], e2[:])
                    e3 = qpool.tile([128, NT], F32, tag="e3", name="e3")
                    nc.gpsimd.tensor_mul(e3[:], rotc8_t[:, 0, :, ci], qinim[:])
                    e4 = qpool.tile([128, NT], F32, tag="e4", name="e4")
                    nc.gpsimd.tensor_mul(e4[:], rotc8_t[:, 1, :, ci], qinre[:])
                    nc.gpsimd.tensor_add(dim[:], e3[:], e4[:])
                    deltas[(p, ci)] = (dre, dim)

            def emit_prep_bias(p, g):
                # a3 = stash + delta; split: re nt0-1 DVE, re nt2-3 Act, im Pool
                a3re = bpool.tile([128, NT, GL], H, tag="a3re", name="a3re", bufs=1)
                a3im = bpool.tile([128, NT, GL], H, tag="a3im", name="a3im", bufs=1)
                for c in range(2):
                    ci = 2 * g + c
                    dre, dim = deltas[(p, ci)]
                    cr = slice(c * 256, (c + 1) * 256)
                    for ntile in range(NT):
                        st = stash[p][0][:, ntile, ci * 256:(ci + 1) * 256]
                        if ntile < 2:
                            nc.vector.tensor_scalar(
                                a3re[:, ntile, cr], st,
                                dre[:, ntile:ntile + 1], None, ADD)
                        else:
                            nc.scalar.activation(
                                a3re[:, ntile, cr], st, CP,
                                bias=dre[:, ntile:ntile + 1])
                    for ntile in range(NT):
                        st = stash[p][1][:, ntile, ci * 256:(ci + 1) * 256]
                        nc.gpsimd.tensor_scalar(
                            a3im[:, ntile, cr], st,
                            dim[:, ntile:ntile + 1], None, ADD)
                return a3re, a3im

            def emit_prep_comb(p, g, a3re, a3im):
                p1 = bpool.tile([128, NT, GL], H, tag="p1", name="p1", bufs=1)
                nc.vector.tensor_mul(p1[:], a3re[:], cmb_t[:, 0, :, :])
                p2 = bpool.tile([128, NT, GL], H, tag="p2", name="p2", bufs=1)
                nc.vector.tensor_mul(p2[:], a3im[:], cmb_t[:, 1, :, :])
                hT3 = bpool.tile([128, NT, GL], H, tag="h", name="hT3", bufs=3)
                nc.vector.tensor_sub(hT3[:], p1[:], p2[:])
                hts[(p, g)] = hT3

            def emit_B_proj(p, g):
                hT3 = hts.pop((p, g))
                for mt in range(8):
                    y_ps = psy.tile([128, GL], F32, tag="y", name="y_ps")
                    for kt in range(4):
                        nc.tensor.matmul(
                            y_ps[:],
                            cwT[kt][:, mt * 128:(mt + 1) * 128],
                            hT3[:, kt, :],
                            start=(kt == 0), stop=(kt == 3))
                    yc = bpool.tile([128, GL], H, tag="yc", name="yc", bufs=4)
                    nc.scalar.activation(yc[:], y_ps[:], CP)
                    yo = bpool.tile([128, GL], H, tag="yo", name="yo", bufs=6)
                    nc.vector.tensor_add(yo[:], yc[:],
                                         uP[p][mt][:, g * GL:(g + 1) * GL])
                    nc.sync.dma_start(
                        yT_d[mt * 128:(mt + 1) * 128,
                             p * PIECE + g * GL:p * PIECE + (g + 1) * GL],
                        yo[:])

            # ================= emission schedule =================
            def zero_q():
                qre = qpool.tile([128, NT], F32, tag="qre", name="qre")
                nc.vector.memset(qre[:], 0.0)
                qim = qpool.tile([128, NT], F32, tag="qim", name="qim")
                nc.vector.memset(qim[:], 0.0)
                return [qre, qim]

            q = zero_q()
            for g in range(NGRP):
                q = emit_A_group(0, g, q)
            qfin[0] = q
            sb0 = emit_coll_pack(0)
            rd0 = emit_coll_comm(0, sb0)
            emit_recv_sbuf(0, rd0, nc.gpsimd)
            cmb_t, rotc8_t = emit_B_weights()

            q = zero_q()
            q = emit_A_group(1, 0, q)
            q = emit_A_group(1, 1, q)
            emit_qin(0)
            emit_prep_delta(0, 0)
            q = emit_A_group(1, 2, q)
            a3_00 = emit_prep_bias(0, 0)
            emit_prep_delta(0, 1)
            q = emit_A_group(1, 3, q)
            qfin[1] = q
            sb1 = emit_coll_pack(1)
            emit_prep_comb(0, 0, *a3_00)
            a3_01 = emit_prep_bias(0, 1)
            rd1 = emit_coll_comm(1, sb1)
            emit_prep_comb(0, 1, *a3_01)
            emit_B_proj(0, 0)
            emit_prep_delta(0, 2)
            a3 = emit_prep_bias(0, 2)
            emit_prep_comb(0, 2, *a3)
            emit_B_proj(0, 1)
            emit_recv_sbuf(1, rd1, nc.sync)
            emit_prep_delta(0, 3)
            a3 = emit_prep_bias(0, 3)
            emit_prep_comb(0, 3, *a3)
            emit_B_proj(0, 2)
            emit_qin(1)
            emit_prep_delta(1, 0)
            a3 = emit_prep_bias(1, 0)
            emit_prep_comb(1, 0, *a3)
            emit_B_proj(0, 3)
            emit_prep_delta(1, 1)
            a3 = emit_prep_bias(1, 1)
            emit_prep_comb(1, 1, *a3)
            emit_B_proj(1, 0)
            emit_prep_delta(1, 2)
            a3 = emit_prep_bias(1, 2)
            emit_prep_comb(1, 2, *a3)
            emit_B_proj(1, 1)
            emit_prep_delta(1, 3)
            a3 = emit_prep_bias(1, 3)
            emit_prep_comb(1, 3, *a3)
            emit_B_proj(1, 2)
            emit_B_proj(1, 3)
    nc.compile()
    return nc


def _host_tables(a_params):
    n = STATE
    half = n // 2
    a_full = np.zeros(n)
    a_full[1:half + 1] = a_params.astype(np.float64)
    a_full[half + 1:] = -a_params.astype(np.float64)[::-1][: n - half - 1]
    omega = np.imag(np.fft.fft(a_full))
    theta = -2.0 * np.arctan(omega)          # (512,)
    p128 = np.arange(128)

    c0Sa = np.cos(p128[:, None] * theta[None, :])
    c0Sb = np.cos((p128[:, None] + 128) * theta[None, :])
    ms0Sa = -np.sin(p128[:, None] * theta[None, :])
    ms0Sb = -np.sin((p128[:, None] + 128) * theta[None, :])
    rotc = np.stack([c0Sa, c0Sb, ms0Sa, ms0Sb], axis=1)       # (128, 4, 512)

    tg = np.arange(GL) % 256
    cmb = np.empty((128, 2, NT, GL))
    for nt in range(NT):
        th = theta[128 * nt:128 * (nt + 1)]
        cmb[:, 0, nt, :] = np.cos(th[:, None] * tg[None, :])
        cmb[:, 1, nt, :] = np.sin(th[:, None] * tg[None, :])

    U = np.triu(np.ones((128, 128)))
    UO = np.concatenate([U, np.ones((128, 128))], axis=1)
    ZU = np.concatenate([np.zeros((128, 128)), U], axis=1)

    thNT = theta.reshape(NT, 128).T                            # (128, NT)
    chain = np.stack([np.cos(L * thNT), np.sin(L * thNT)], axis=1)

    cs = np.arange(NCHUNK)
    ang8 = thNT[:, :, None] * (cs[None, None, :] * 256.0)      # (128, NT, 8)
    rotc8 = np.stack([np.cos(ang8), np.sin(ang8)], axis=1)     # (128, 2, NT, 8)

    rho = np.exp(1j * 2048.0 * thNT)                           # (128, NT) complex
    tabs = {
        "rotc": rotc.astype(NPH),
        "cmb": cmb.astype(NPH),
        "UO": UO.astype(NPH),
        "ZU": ZU.astype(NPH),
        "chain": np.ascontiguousarray(chain, dtype=np.float32),
        "rotc8": np.ascontiguousarray(rotc8, dtype=np.float32),
    }
    return tabs, rho


def _masks_for(member, rho):
    ones = np.ones_like(rho.real)
    zeros = np.zeros_like(rho.real)
    if member == 0:
        m0, b = zeros, zeros
        W1, W2 = ones + 0j, rho
    else:
        m0, b = ones, ones
        W1, W2 = rho, rho * rho
    mask = np.stack([m0, b, W1.real, W1.imag, W2.real, W2.imag], axis=1)
    return np.ascontiguousarray(mask, dtype=np.float32)        # (128, 6, NT)


def kernel(u, a_params, B_w, C_w, D, trace=False):
    u = np.asarray(u, dtype=np.float32)
    B_w = np.asarray(B_w, dtype=np.float32)
    C_w = np.asarray(C_w, dtype=np.float32)
    D = np.asarray(D, dtype=np.float32)
    assert np.allclose(D, 1.0), "kernel assumes D == ones (skip shares u tiles)"
    tabs, rho = _host_tables(np.asarray(a_params))

    if "nc" not in _CACHE:
        _CACHE["nc"] = build_nc()
    nc = _CACHE["nc"]

    bwT = np.ascontiguousarray(B_w.T).astype(NPH)              # (1024, 512)
    cwT = np.ascontiguousarray(C_w.T).astype(NPH)              # (512, 1024)

    in_maps = []
    for core in range(8):
        s, m = core // 2, core % 2
        qa, qb = m, 2 + m
        uT = np.concatenate(
            [u[s, qa * PIECE:(qa + 1) * PIECE, :].T,
             u[s, qb * PIECE:(qb + 1) * PIECE, :].T], axis=1)   # (1024, 4096)
        mp = {
            "uT": np.ascontiguousarray(uT).astype(NPH),
            "bwT": bwT,
            "cwT": cwT,
            "mask": _masks_for(m, rho),
        }
        mp.update(tabs)
        in_maps.append(mp)

    res = bass_utils.run_bass_kernel_spmd(
        nc, in_maps, core_ids=list(range(8)), trace=trace)
    y = np.empty((BATCH, SEQ, D_MODEL), dtype=np.float32)
    for core in range(8):
        s, m = core // 2, core % 2
        qa, qb = m, 2 + m
        yT = res.results[core]["yT"].astype(np.float32)        # (1024, 4096) fp16
        y[s, qa * PIECE:(qa + 1) * PIECE, :] = yT[:, 0:PIECE].T
        y[s, qb * PIECE:(qb + 1) * PIECE, :] = yT[:, PIECE:2 * PIECE].T
    _CACHE["last_res"] = res
    return y


# revision 22
# speedup vs baseline: 1.1827x; 1.0084x over previous
"""Trainium2 Bass kernel for CayleyCirculantSSMLayer (seq-sharded, 8 cores).

Math: lambda_j = e^{i*theta_j} with theta = -2*arctan(omega).  The causal conv
h[t] = sum_{s<=t} cos((t-s)theta) Bu[s] factors through a chunk-rotated
cumulative sum (chunks of L=256 rows): within a chunk A = Q + cumsum(rotBu),
h = cos(sigma*theta) (.) A_re - sin(sigma*theta) (.) A_im, carry
Q' = lambda^L (Q + R) chained across chunks.

Sharding: each core owns TWO quarter-sample pieces (sample k//2, quarters
k%2 and 2+k%2), so a sample's 4 quarters live on the core pair {2s, 2s+1}.
The causal carry entering quarter q is a linear combination of the
zero-initialized quarter totals T_r (r<q), exchanged between the pair via
tiny (4KB) AllGather collectives that hide under compute:

    A(p0) -> [allgather T(p0)] -> A(p1) -> [allgather T(p1)] -> B(p0) -> B(p1)

Phase A computes Bu (fp16 matmul), rotation, chunk cumsums (triangular
matmul), stashes locally-biased A in fp16, chains the local carry.  Phase B
applies the late carry correction delta_c = lambda^{256c} * Q_in as a
per-chunk bias, combines to h, projects through C (fp16 matmul), adds the
skip, and streams the output in fp16.  B-group prep (bias/combine) is
software-pipelined into the preceding emission so the PE never waits on it.
"""
import sys
import numpy as np

for p in ("/opt/trn_rl_repo",):
    if p not in sys.path:
        sys.path.insert(0, p)

from concourse import bass, bacc, mybir, tile
from concourse import bass_utils

D_MODEL = 1024
STATE = 512
BATCH = 4
SEQ = 8192
PIECE = 2048              # rows per piece (quarter sample)
L = 256                   # carry-chunk length
GL = 512                  # group length (2 chunks)
NGRP = PIECE // GL        # 4 groups per piece
NCHUNK = PIECE // L       # 8 chunks per piece
NT = 4                    # state n-tiles of 128
F32 = mybir.dt.float32
H = mybir.dt.float16
NPH = np.float16

_CACHE = {}


def build_nc():
    nc = bacc.Bacc(None, target_bir_lowering=False, num_devices=8)
    uT_d = nc.dram_tensor("uT", [D_MODEL, 2 * PIECE], H, kind="ExternalInput")
    bwT_d = nc.dram_tensor("bwT", [D_MODEL, STATE], H, kind="ExternalInput")
    cwT_d = nc.dram_tensor("cwT", [STATE, D_MODEL], H, kind="ExternalInput")
    rotc_d = nc.dram_tensor("rotc", [128, 4, STATE], H, kind="ExternalInput")
    cmb_d = nc.dram_tensor("cmb", [128, 2, NT, GL], H, kind="ExternalInput")
    UO_d = nc.dram_tensor("UO", [128, 256], H, kind="ExternalInput")
    ZU_d = nc.dram_tensor("ZU", [128, 256], H, kind="ExternalInput")
    chain_d = nc.dram_tensor("chain", [128, 2, NT], F32, kind="ExternalInput")
    rotc8_d = nc.dram_tensor("rotc8", [128, 2, NT, NCHUNK], F32, kind="ExternalInput")
    mask_d = nc.dram_tensor("mask", [128, 6, NT], F32, kind="ExternalInput")
    yT_d = nc.dram_tensor("yT", [D_MODEL, 2 * PIECE], H, kind="ExternalOutput")

    CP = mybir.ActivationFunctionType.Identity
    ADD = mybir.AluOpType.add
    GROUPS = [[0, 1], [2, 3], [4, 5], [6, 7]]

    with tile.TileContext(nc) as tc:
        with (
            tc.tile_pool(name="c", bufs=1) as cpool,
            tc.tile_pool(name="u", bufs=2) as upool,
            tc.tile_pool(name="q", bufs=3) as qpool,
            tc.tile_pool(name="dram", bufs=1, space="DRAM") as dpool,
            tc.tile_pool(name="psbu", bufs=2, space="PSUM") as psbu,
            tc.tile_pool(name="psw", bufs=4, space="PSUM") as psw,
            tc.tile_pool(name="psy", bufs=2, space="PSUM") as psy,
            tc.tile_pool(name="pa", bufs=1) as apool,
            tc.tile_pool(name="pv", bufs=3) as vpool,
            tc.tile_pool(name="pb", bufs=2) as bpool,
        ):
            # ---------- A-phase weights (gpsimd queue, loaded first) ----------
            bwT = []
            for k in range(8):
                t = apool.tile([128, STATE], H, tag=f"bw{k}", name=f"bw{k}")
                nc.gpsimd.dma_start(t[:], bwT_d[k * 128:(k + 1) * 128, :])
                bwT.append(t)
            rotc_t = apool.tile([128, 4, STATE], H, tag="rotc", name="rotc")
            nc.gpsimd.dma_start(rotc_t[:], rotc_d[:, :, :])
            UO_t = apool.tile([128, 256], H, tag="UO", name="UO")
            nc.gpsimd.dma_start(UO_t[:], UO_d[:, :])
            ZU_t = apool.tile([128, 256], H, tag="ZU", name="ZU")
            nc.gpsimd.dma_start(ZU_t[:], ZU_d[:, :])
            chain_t = cpool.tile([128, 2, NT], F32, tag="chain", name="chain")
            nc.gpsimd.dma_start(chain_t[:], chain_d[:, :, :])
            mask_t = cpool.tile([128, 6, NT], F32, tag="mask", name="mask")
            nc.gpsimd.dma_start(mask_t[:], mask_d[:, :, :])

            # ---------- all u loads up front (SP queue) ----------
            uP = [[None] * 8, [None] * 8]
            for p in range(2):
                for k in range(8):
                    uP[p][k] = upool.tile([128, PIECE], H, tag=f"ut{k}",
                                          name=f"ut{k}")
                if p == 0:
                    # group-ordered small loads so A(0,g) starts ASAP
                    for g in range(NGRP):
                        for k in range(8):
                            nc.sync.dma_start(
                                uP[p][k][:, g * GL:(g + 1) * GL],
                                uT_d[k * 128:(k + 1) * 128,
                                     p * PIECE + g * GL:p * PIECE + (g + 1) * GL])
                else:
                    for k in range(8):
                        nc.sync.dma_start(
                            uP[p][k][:, 0:GL],
                            uT_d[k * 128:(k + 1) * 128, p * PIECE:p * PIECE + GL])
                    for k in range(8):
                        nc.sync.dma_start(
                            uP[p][k][:, GL:PIECE],
                            uT_d[k * 128:(k + 1) * 128,
                                 p * PIECE + GL:(p + 1) * PIECE])

            # ---------- persistent state ----------
            stash = [[cpool.tile([128, NT, PIECE], H, tag=f"st{p}{co}",
                                 name=f"st{p}{co}")
                      for co in range(2)] for p in range(2)]
            g_r0 = []
            g_r1 = []
            for p in range(2):
                g_r0.append(cpool.tile([128, 8], F32, tag=f"gr0{p}", name=f"gr0{p}"))
                g_r1.append(cpool.tile([128, 8], F32, tag=f"gr1{p}", name=f"gr1{p}"))
            cwT = []
            cmb_t = None
            rotc8_t = None
            qfin = [None, None]          # final chain tiles per piece
            qin = [None, None]           # (re, im) per piece
            hts = {}                     # (p, g) -> hT3 tile

            # ================= emission helpers =================
            def emit_A_group(p, g, q):
                v = []
                for s4 in range(4):
                    bu_ps = psbu.tile([128, STATE], F32, tag="bu", name="bu")
                    for k in range(8):
                        nc.tensor.matmul(
                            bu_ps[:],
                            uP[p][k][:, g * GL + s4 * 128:g * GL + (s4 + 1) * 128],
                            bwT[k][:],
                            start=(k == 0), stop=(k == 7))
                    sub = s4 % 2
                    vre = vpool.tile([128, STATE], H, tag="vre", name="vre")
                    nc.vector.tensor_mul(vre[:], bu_ps[:], rotc_t[:, sub, :])
                    vim = vpool.tile([128, STATE], H, tag="vim", name="vim")
                    nc.vector.tensor_mul(vim[:], bu_ps[:], rotc_t[:, 2 + sub, :])
                    v.append((vre, vim))

                for c in range(2):
                    ci = 2 * g + c
                    x0, x1 = v[2 * c], v[2 * c + 1]
                    tc_re = qpool.tile([128, NT], F32, tag="tcre", name="tcre")
                    tc_im = qpool.tile([128, NT], F32, tag="tcim", name="tcim")
                    tcs = [tc_re, tc_im]
                    for co in range(2):
                        d0, d1 = x0[co], x1[co]
                        for pr in range(2):
                            w_ps = psw.tile([128, GL], F32, tag="w", name="w")
                            for half in range(2):
                                ntile = 2 * pr + half
                                colr = slice(half * 256, half * 256 + 256)
                                nc.tensor.matmul(
                                    w_ps[:, colr],
                                    d0[:, ntile * 128:(ntile + 1) * 128],
                                    UO_t[:], start=True, stop=False)
                                nc.tensor.matmul(
                                    w_ps[:, colr],
                                    d1[:, ntile * 128:(ntile + 1) * 128],
                                    ZU_t[:], start=False, stop=True)
                            for half in range(2):
                                ntile = 2 * pr + half
                                nc.scalar.activation(
                                    stash[p][co][:, ntile, ci * 256:(ci + 1) * 256],
                                    w_ps[:, half * 256:half * 256 + 256],
                                    CP, bias=q[co][:, ntile:ntile + 1])
                            nc.vector.tensor_add(
                                tcs[co][:, 2 * pr:2 * pr + 2],
                                w_ps[:, 255::256],
                                q[co][:, 2 * pr:2 * pr + 2])
                    m1 = qpool.tile([128, NT], F32, tag="m1", name="m1")
                    nc.vector.tensor_mul(m1[:], tc_re[:], chain_t[:, 0, :])
                    m2 = qpool.tile([128, NT], F32, tag="m2", name="m2")
                    nc.vector.tensor_mul(m2[:], tc_im[:], chain_t[:, 1, :])
                    qre = qpool.tile([128, NT], F32, tag="qre", name="qre")
                    nc.vector.tensor_sub(qre[:], m1[:], m2[:])
                    m3 = qpool.tile([128, NT], F32, tag="m3", name="m3")
                    nc.vector.tensor_mul(m3[:], tc_im[:], chain_t[:, 0, :])
                    m4 = qpool.tile([128, NT], F32, tag="m4", name="m4")
                    nc.vector.tensor_mul(m4[:], tc_re[:], chain_t[:, 1, :])
                    qim = qpool.tile([128, NT], F32, tag="qim", name="qim")
                    nc.vector.tensor_add(qim[:], m3[:], m4[:])
                    q = [qre, qim]
                return q

            def emit_coll_pack(p):
                send_sb = qpool.tile([128, 8], F32, tag="send", name="send")
                nc.vector.tensor_copy(send_sb[:, 0:4], qfin[p][0][:])
                nc.vector.tensor_copy(send_sb[:, 4:8], qfin[p][1][:])
                return send_sb

            def emit_coll_comm(p, send_sb):
                send_d = dpool.tile([128, 8], F32, tag=f"send{p}", name=f"send{p}")
                recv_d = dpool.tile([256, 8], F32, tag=f"recv{p}", name=f"recv{p}")
                nc.gpsimd.dma_start(send_d[:], send_sb[:])
                nc.gpsimd.collective_compute(
                    "AllGather",
                    mybir.AluOpType.bypass,
                    replica_groups=GROUPS,
                    ins=[send_d[:].opt()],
                    outs=[recv_d[:].opt()],
                )
                return recv_d

            def emit_recv_sbuf(p, recv_d, eng):
                eng.dma_start(g_r0[p][:], recv_d[0:128, :])
                eng.dma_start(g_r1[p][:], recv_d[128:256, :])

            def emit_B_weights():
                for k in range(4):
                    t = cpool.tile([128, D_MODEL], H, tag=f"cw{k}", name=f"cw{k}")
                    nc.gpsimd.dma_start(t[:], cwT_d[k * 128:(k + 1) * 128, :])
                    cwT.append(t)
                cmb = cpool.tile([128, 2, NT, GL], H, tag="cmb", name="cmb")
                nc.gpsimd.dma_start(cmb[:], cmb_d[:, :, :, :])
                r8 = cpool.tile([128, 2, NT, NCHUNK], F32, tag="rotc8", name="rotc8")
                nc.gpsimd.dma_start(r8[:], rotc8_d[:, :, :, :])
                return cmb, r8

            def emit_qin(p):
                qinre = qpool.tile([128, NT], F32, tag="qinre", name="qinre")
                qinim = qpool.tile([128, NT], F32, tag="qinim", name="qinim")
                if p == 0:
                    nc.vector.tensor_mul(qinre[:], mask_t[:, 0, :], g_r0[0][:, 0:4])
                    nc.vector.tensor_mul(qinim[:], mask_t[:, 0, :], g_r0[0][:, 4:8])
                else:
                    acc_re = None
                    acc_im = None
                    terms = [
                        (1, g_r0[1], 0, 1.0), (2, g_r1[0], 0, 1.0),
                        (3, g_r1[0], 4, -1.0), (4, g_r0[0], 0, 1.0),
                        (5, g_r0[0], 4, -1.0),
                    ]
                    # re part: b*G2re + W1re*G1[1]re - W1im*G1[1]im
                    #          + W2re*G1[0]re - W2im*G1[0]im
                    for i, (mi, src, off, sgn) in enumerate(terms):
                        t = qpool.tile([128, NT], F32, tag=f"x{i}", name=f"x{i}")
                        nc.vector.tensor_mul(t[:], mask_t[:, mi, :],
                                             src[:, off:off + 4])
                        if acc_re is None:
                            acc_re = t
                        else:
                            nt_ = qpool.tile([128, NT], F32, tag=f"xa{i}",
                                             name=f"xa{i}")
                            if sgn > 0:
                                nc.vector.tensor_add(nt_[:], acc_re[:], t[:])
                            else:
                                nc.vector.tensor_sub(nt_[:], acc_re[:], t[:])
                            acc_re = nt_
                    terms_im = [
                        (1, g_r0[1], 4), (2, g_r1[0], 4), (3, g_r1[0], 0),
                        (4, g_r0[0], 4), (5, g_r0[0], 0),
                    ]
                    for i, (mi, src, off) in enumerate(terms_im):
                        t = qpool.tile([128, NT], F32, tag=f"y{i}", name=f"y{i}")
                        nc.vector.tensor_mul(t[:], mask_t[:, mi, :],
                                             src[:, off:off + 4])
                        if acc_im is None:
                            acc_im = t
                        else:
                            nt_ = qpool.tile([128, NT], F32, tag=f"ya{i}",
                                             name=f"ya{i}")
                            nc.vector.tensor_add(nt_[:], acc_im[:], t[:])
                            acc_im = nt_
                    nc.vector.tensor_copy(qinre[:], acc_re[:])
                    nc.vector.tensor_copy(qinim[:], acc_im[:])
                qin[p] = (qinre, qinim)
                return qinre, qinim

            deltas = {}

            def emit_prep_delta(p, g):
                # delta_c = lam^{256 ci} * Q_in, on gpsimd (Pool)
                qinre, qinim = qin[p]
                for c in range(2):
                    ci = 2 * g + c
                    dre = qpool.tile([128, NT], F32, tag="dre", name="dre", bufs=4)
                    dim = qpool.tile([128, NT], F32, tag="dim", name="dim", bufs=4)
                    e1 = qpool.tile([128, NT], F32, tag="e1", name="e1")
                    nc.gpsimd.tensor_mul(e1[:], rotc8_t[:, 0, :, ci], qinre[:])
                    e2 = qpool.tile([128, NT], F32, tag="e2", name="e2")
                    nc.gpsimd.tensor_mul(e2[:], rotc8_t[:, 1, :, ci], qinim[:])
                    nc.gpsimd.tensor_sub(dre[:], e1[:], e2[:])
                    e3 = qpool.tile([128, NT], F32, tag="e3", name="e3")
                    nc.gpsimd.tensor_mul(e3[:], rotc8_t[:, 0, :, ci], qinim[:])
                    e4 = qpool.tile([128, NT], F32, tag="e4", name="e4")
                    nc.gpsimd.tensor_mul(e4[:], rotc8_t[:, 1, :, ci], qinre[:])
                    nc.gpsimd.tensor_add(dim[:], e3[:], e4[:])
                    deltas[(p, ci)] = (dre, dim)

            def emit_prep_bias(p, g):
                # a3 = stash + delta; split: re nt0-1 DVE, re nt2-3 Act, im Pool
                a3re = bpool.tile([128, NT, GL], H, tag="a3re", name="a3re", bufs=1)
                a3im = bpool.tile([128, NT, GL], H, tag="a3im", name="a3im", bufs=1)
                for c in range(2):
                    ci = 2 * g + c
                    dre, dim = deltas[(p, ci)]
                    cr = slice(c * 256, (c + 1) * 256)
                    for ntile in range(NT):
                        st = stash[p][0][:, ntile, ci * 256:(ci + 1) * 256]
                        if ntile < 2:
                            nc.vector.tensor_scalar(
                                a3re[:, ntile, cr], st,
                                dre[:, ntile:ntile + 1], None, ADD)
                        else:
                            nc.scalar.activation(
                                a3re[:, ntile, cr], st, CP,
                                bias=dre[:, ntile:ntile + 1])
                    for ntile in range(NT):
                        st = stash[p][1][:, ntile, ci * 256:(ci + 1) * 256]
                        nc.gpsimd.tensor_scalar(
                            a3im[:, ntile, cr], st,
                            dim[:, ntile:ntile + 1], None, ADD)
                return a3re, a3im

            def emit_prep_comb(p, g, a3re, a3im):
                p1 = bpool.tile([128, NT, GL], H, tag="p1", name="p1", bufs=1)
                nc.vector.tensor_mul(p1[:], a3re[:], cmb_t[:, 0, :, :])
                p2 = bpool.tile([128, NT, GL], H, tag="p2", name="p2", bufs=1)
                nc.vector.tensor_mul(p2[:], a3im[:], cmb_t[:, 1, :, :])
                hT3 = bpool.tile([128, NT, GL], H, tag="h", name="hT3", bufs=3)
                nc.vector.tensor_sub(hT3[:], p1[:], p2[:])
                hts[(p, g)] = hT3

            def emit_B_proj(p, g):
                # skip path (D*u) is added on the host; device emits proj only
                hT3 = hts.pop((p, g))
                for mt in range(8):
                    y_ps = psy.tile([128, GL], F32, tag="y", name="y_ps")
                    for kt in range(4):
                        nc.tensor.matmul(
                            y_ps[:],
                            cwT[kt][:, mt * 128:(mt + 1) * 128],
                            hT3[:, kt, :],
                            start=(kt == 0), stop=(kt == 3))
                    yc = bpool.tile([128, GL], H, tag="yc", name="yc", bufs=6)
                    nc.scalar.activation(yc[:], y_ps[:], CP)
                    nc.sync.dma_start(
                        yT_d[mt * 128:(mt + 1) * 128,
                             p * PIECE + g * GL:p * PIECE + (g + 1) * GL],
                        yc[:])

            # ================= emission schedule =================
            def zero_q():
                qre = qpool.tile([128, NT], F32, tag="qre", name="qre")
                nc.vector.memset(qre[:], 0.0)
                qim = qpool.tile([128, NT], F32, tag="qim", name="qim")
                nc.vector.memset(qim[:], 0.0)
                return [qre, qim]

            q = zero_q()
            for g in range(NGRP):
                q = emit_A_group(0, g, q)
            qfin[0] = q
            sb0 = emit_coll_pack(0)
            rd0 = emit_coll_comm(0, sb0)
            emit_recv_sbuf(0, rd0, nc.gpsimd)
            cmb_t, rotc8_t = emit_B_weights()

            q = zero_q()
            q = emit_A_group(1, 0, q)
            q = emit_A_group(1, 1, q)
            emit_qin(0)
            emit_prep_delta(0, 0)
            q = emit_A_group(1, 2, q)
            a3_00 = emit_prep_bias(0, 0)
            emit_prep_delta(0, 1)
            q = emit_A_group(1, 3, q)
            qfin[1] = q
            sb1 = emit_coll_pack(1)
            emit_prep_comb(0, 0, *a3_00)
            a3_01 = emit_prep_bias(0, 1)
            rd1 = emit_coll_comm(1, sb1)
            emit_prep_comb(0, 1, *a3_01)
            emit_B_proj(0, 0)
            emit_prep_delta(0, 2)
            a3 = emit_prep_bias(0, 2)
            emit_prep_comb(0, 2, *a3)
            emit_B_proj(0, 1)
            emit_recv_sbuf(1, rd1, nc.sync)
            emit_prep_delta(0, 3)
            a3 = emit_prep_bias(0, 3)
            emit_prep_comb(0, 3, *a3)
            emit_B_proj(0, 2)
            emit_qin(1)
            emit_prep_delta(1, 0)
            a3 = emit_prep_bias(1, 0)
            emit_prep_comb(1, 0, *a3)
            emit_B_proj(0, 3)
            emit_prep_delta(1, 1)
            a3 = emit_prep_bias(1, 1)
            emit_prep_comb(1, 1, *a3)
            emit_B_proj(1, 0)
            emit_prep_delta(1, 2)
            a3 = emit_prep_bias(1, 2)
            emit_prep_comb(1, 2, *a3)
            emit_B_proj(1, 1)
            emit_prep_delta(1, 3)
            a3 = emit_prep_bias(1, 3)
            emit_prep_comb(1, 3, *a3)
            emit_B_proj(1, 2)
            emit_B_proj(1, 3)
    nc.compile()
    return nc


def _host_tables(a_params):
    n = STATE
    half = n // 2
    a_full = np.zeros(n)
    a_full[1:half + 1] = a_params.astype(np.float64)
    a_full[half + 1:] = -a_params.astype(np.float64)[::-1][: n - half - 1]
    omega = np.imag(np.fft.fft(a_full))
    theta = -2.0 * np.arctan(omega)          # (512,)
    p128 = np.arange(128)

    c0Sa = np.cos(p128[:, None] * theta[None, :])
    c0Sb = np.cos((p128[:, None] + 128) * theta[None, :])
    ms0Sa = -np.sin(p128[:, None] * theta[None, :])
    ms0Sb = -np.sin((p128[:, None] + 128) * theta[None, :])
    rotc = np.stack([c0Sa, c0Sb, ms0Sa, ms0Sb], axis=1)       # (128, 4, 512)

    tg = np.arange(GL) % 256
    cmb = np.empty((128, 2, NT, GL))
    for nt in range(NT):
        th = theta[128 * nt:128 * (nt + 1)]
        cmb[:, 0, nt, :] = np.cos(th[:, None] * tg[None, :])
        cmb[:, 1, nt, :] = np.sin(th[:, None] * tg[None, :])

    U = np.triu(np.ones((128, 128)))
    UO = np.concatenate([U, np.ones((128, 128))], axis=1)
    ZU = np.concatenate([np.zeros((128, 128)), U], axis=1)

    thNT = theta.reshape(NT, 128).T                            # (128, NT)
    chain = np.stack([np.cos(L * thNT), np.sin(L * thNT)], axis=1)

    cs = np.arange(NCHUNK)
    ang8 = thNT[:, :, None] * (cs[None, None, :] * 256.0)      # (128, NT, 8)
    rotc8 = np.stack([np.cos(ang8), np.sin(ang8)], axis=1)     # (128, 2, NT, 8)

    rho = np.exp(1j * 2048.0 * thNT)                           # (128, NT) complex
    tabs = {
        "rotc": rotc.astype(NPH),
        "cmb": cmb.astype(NPH),
        "UO": UO.astype(NPH),
        "ZU": ZU.astype(NPH),
        "chain": np.ascontiguousarray(chain, dtype=np.float32),
        "rotc8": np.ascontiguousarray(rotc8, dtype=np.float32),
    }
    return tabs, rho


def _masks_for(member, rho):
    ones = np.ones_like(rho.real)
    zeros = np.zeros_like(rho.real)
    if member == 0:
        m0, b = zeros, zeros
        W1, W2 = ones + 0j, rho
    else:
        m0, b = ones, ones
        W1, W2 = rho, rho * rho
    mask = np.stack([m0, b, W1.real, W1.imag, W2.real, W2.imag], axis=1)
    return np.ascontiguousarray(mask, dtype=np.float32)        # (128, 6, NT)


def kernel(u, a_params, B_w, C_w, D, trace=False):
    u = np.asarray(u, dtype=np.float32)
    B_w = np.asarray(B_w, dtype=np.float32)
    C_w = np.asarray(C_w, dtype=np.float32)
    D = np.asarray(D, dtype=np.float32)
    tabs, rho = _host_tables(np.asarray(a_params))

    if "nc" not in _CACHE:
        _CACHE["nc"] = build_nc()
    nc = _CACHE["nc"]

    bwT = np.ascontiguousarray(B_w.T).astype(NPH)              # (1024, 512)
    cwT = np.ascontiguousarray(C_w.T).astype(NPH)              # (512, 1024)

    in_maps = []
    for core in range(8):
        s, m = core // 2, core % 2
        qa, qb = m, 2 + m
        uT = np.concatenate(
            [u[s, qa * PIECE:(qa + 1) * PIECE, :].T,
             u[s, qb * PIECE:(qb + 1) * PIECE, :].T], axis=1)   # (1024, 4096)
        mp = {
            "uT": np.ascontiguousarray(uT).astype(NPH),
            "bwT": bwT,
            "cwT": cwT,
            "mask": _masks_for(m, rho),
        }
        mp.update(tabs)
        in_maps.append(mp)

    res = bass_utils.run_bass_kernel_spmd(
        nc, in_maps, core_ids=list(range(8)), trace=trace)
    y = np.empty((BATCH, SEQ, D_MODEL), dtype=np.float32)
    for core in range(8):
        s, m = core // 2, core % 2
        qa, qb = m, 2 + m
        yT = res.results[core]["yT"].astype(np.float32)        # (1024, 4096) fp16
        y[s, qa * PIECE:(qa + 1) * PIECE, :] = (
            yT[:, 0:PIECE].T + D * u[s, qa * PIECE:(qa + 1) * PIECE, :])
        y[s, qb * PIECE:(qb + 1) * PIECE, :] = (
            yT[:, PIECE:2 * PIECE].T + D * u[s, qb * PIECE:(qb + 1) * PIECE, :])
    _CACHE["last_res"] = res
    return y
